# revision 42
# baseline (speedup 1.0000x reference)
"""Trainium2 Bass kernel for a dense transformer block (B=4, N=2048, C=768, H=12).

Sharding: 8 cores = 4 batches x 2 sequence halves. Each core receives its
batch's rows rolled so its own 1024 query rows are rows 0:1023 (softmax is
permutation-invariant over keys, so key order doesn't matter). Each core
computes LN1 over all 2048 rows, K/V per 4-head group and attention + MLP for
its own 1024 rows, returning a [1024, 768] output slice. No collectives.

All matmuls run in float32r (full PE rate, ~1e-4 rounding). Dataflow stays in
transposed [channel, token] layouts so contractions land on SBUF partitions.
Softmax denominators ride the values matmul as an appended ones-column; the
per-query 1/denom is applied by folding diag(r) into the PE transpose that
restores each head's [channel, token] layout. SBUF is managed as tag-chained
slots (five 24KB/partition slots rotate through the phase-chained tensors).
"""

import numpy as np

B, N, C = 4, 2048, 768
H, DH = 12, 64
HID = 4 * C
SCALE = DH ** -0.5
EPS = 1e-5

P = 128
CT = C // P          # 6
NT = N // P          # 16
NO = N // 2          # 1024 own rows
NOT_ = NO // P       # 8
HT = HID // P        # 24


def _build_bass():
    import concourse.bass as bass
    import concourse.tile as tile
    from concourse import bacc, mybir
    from concourse.masks import make_identity
    from concourse.alu_op_type import AluOpType as A

    F32 = mybir.dt.float32
    F32R = mybir.dt.float32r
    AF = mybir.ActivationFunctionType

    nc = bacc.Bacc("TRN2", target_bir_lowering=False, num_swdge_queues=4)

    xb = nc.dram_tensor("xb", [N, C], F32, kind="ExternalInput")
    w_qkv = nc.dram_tensor("w_qkv", [C, 3 * C], F32, kind="ExternalInput")
    w_proj = nc.dram_tensor("w_proj", [C, C], F32, kind="ExternalInput")
    w_fc1 = nc.dram_tensor("w_fc1", [C, HID], F32, kind="ExternalInput")
    w_fc2 = nc.dram_tensor("w_fc2", [HID, C], F32, kind="ExternalInput")
    ln1_g = nc.dram_tensor("ln1_g", [C], F32, kind="ExternalInput")
    ln1_b = nc.dram_tensor("ln1_b", [C], F32, kind="ExternalInput")
    ln2_g = nc.dram_tensor("ln2_g", [C], F32, kind="ExternalInput")
    ln2_b = nc.dram_tensor("ln2_b", [C], F32, kind="ExternalInput")
    b_proj = nc.dram_tensor("b_proj", [C], F32, kind="ExternalInput")
    b_fc1 = nc.dram_tensor("b_fc1", [HID], F32, kind="ExternalInput")
    b_fc2 = nc.dram_tensor("b_fc2", [C], F32, kind="ExternalInput")
    out = nc.dram_tensor("out", [NO, C], F32, kind="ExternalOutput")

    dma = nc.gpsimd.dma_start

    with tile.TileContext(nc) as tc:
        consts = tc.alloc_tile_pool(name="consts", bufs=1)
        pbc = tc.alloc_tile_pool(name="pbc", bufs=1)        # LN gamma/beta bcast
        psmall = tc.alloc_tile_pool(name="psmall", bufs=1)  # denominators etc.
        work = tc.alloc_tile_pool(name="work", bufs=2)
        main = tc.alloc_tile_pool(name="main", bufs=1)      # five 24KB slots
        stream = tc.alloc_tile_pool(name="stream", bufs=2)  # weights/exp stream
        pkt = tc.alloc_tile_pool(name="pkt", bufs=1)        # K^T per pair

        ident = consts.tile([P, P], F32)
        make_identity(nc, ident)
        ident_r = consts.tile([P, P], F32R)
        nc.vector.tensor_copy(ident_r, ident)
        eps_t = consts.tile([P, 1], F32)
        nc.vector.memset(eps_t, EPS)
        ones_col = consts.tile([P, 1], F32)
        nc.vector.memset(ones_col, 1.0)
        bpT = consts.tile([P, CT], F32)
        dma(out=bpT, in_=b_proj[:].rearrange("(t p) -> p t", p=P))
        bf1T = consts.tile([P, HT], F32)
        dma(out=bf1T, in_=b_fc1[:].rearrange("(t p) -> p t", p=P))
        bf2T = consts.tile([P, CT], F32)
        dma(out=bf2T, in_=b_fc2[:].rearrange("(t p) -> p t", p=P))

        def layernorm_tile(x_t, g_bc, b_bc):
            st = work.tile([P, 3, 6], F32, tag="ln_st")
            for s in range(3):
                nc.vector.bn_stats(out=st[:, s, :], in_=x_t[:, s * 256:(s + 1) * 256])
            mv = work.tile([P, 2], F32, tag="ln_mv")
            nc.vector.bn_aggr(out=mv, in_=st)
            lnv = work.tile([P, 1], F32, tag="ln_lnv")
            nc.scalar.activation(out=lnv, in_=mv[:, 1:2], func=AF.Ln, bias=eps_t)
            r = work.tile([P, 1], F32, tag="ln_r")
            nc.scalar.activation(out=r, in_=lnv, func=AF.Exp, scale=-0.5)
            h = work.tile([P, C], F32, tag="ln_h")
            nc.vector.tensor_scalar(out=h, in0=x_t, scalar1=mv[:, 0:1], scalar2=r,
                                    op0=A.subtract, op1=A.mult)
            nc.vector.tensor_tensor(out=h, in0=h, in1=g_bc, op=A.mult)
            nc.vector.tensor_tensor(out=h, in0=h, in1=b_bc, op=A.add)
            return h

        def transpose_768(src, dst_view, ps_pool, ps_tag="tr"):
            tp = ps_pool.tile([P, C], F32, tag=ps_tag)
            for t in range(CT):
                nc.tensor.transpose(tp[:, t * P:(t + 1) * P],
                                    src[:, t * P:(t + 1) * P], ident)
            nc.vector.tensor_copy(out=dst_view,
                                  in_=tp[:].rearrange("p (t n) -> p t n", t=CT))

        # ---------------- Phase A: LN1 + transpose -> hT0/hT1 [128, 3, 2048] f32r
        hT0 = main.tile([P, 3, N], F32R, tag="S1")
        hT1 = main.tile([P, 3, N], F32R, tag="S2")

        def hts(kt, sl):
            return hT0[:, kt, sl] if kt < 3 else hT1[:, kt - 3, sl]

        g1_bc = pbc.tile([P, C], F32, tag="g_bc")
        dma(out=g1_bc, in_=ln1_g[:].partition_broadcast(P))
        b1_bc = pbc.tile([P, C], F32, tag="b_bc")
        dma(out=b1_bc, in_=ln1_b[:].partition_broadcast(P))
        with tc.tile_pool(name="ps_trA", bufs=2, space="PSUM") as ps_trA:
            for i in range(NT):
                x_t = work.tile([P, C], F32, tag="io")
                dma(out=x_t, in_=xb[i * P:(i + 1) * P, :])
                hg = layernorm_tile(x_t, g1_bc, b1_bc)
                tp = ps_trA.tile([P, C], F32, tag="tr")
                for t in range(CT):
                    nc.tensor.transpose(tp[:, t * P:(t + 1) * P],
                                        hg[:, t * P:(t + 1) * P], ident)
                nc.vector.tensor_copy(
                    out=hT0[:, :, i * P:(i + 1) * P],
                    in_=tp[:, 0:384].rearrange("p (t n) -> p t n", t=3))
                nc.vector.tensor_copy(
                    out=hT1[:, :, i * P:(i + 1) * P],
                    in_=tp[:, 384:768].rearrange("p (t n) -> p t n", t=3))

        # ---------------- Phase B: attention, per group of 4 heads (2 pairs)
        YTraw = main.tile([P, CT, NO], F32, tag="S3")
        den = psmall.tile([H, NO], F32)
        with tc.tile_pool(name="ps_b", bufs=1, space="PSUM") as ps_b, \
             tc.tile_pool(name="ps_y", bufs=1, space="PSUM") as ps_y:
            for pg in range(3):
                # V for heads 4pg..4pg+3, token-major with an appended ones col
                V_g = main.tile([P, NT, 4 * 65], F32R, tag="S5")
                wv = stream.tile([P, CT, 256], F32R, tag="w")
                dma(out=wv, in_=w_qkv[:, 2 * C + 256 * pg:2 * C + 256 * (pg + 1)]
                    .rearrange("(t p) j -> p t j", p=P))
                for i in range(NT):
                    vps = ps_b.tile([P, 256], F32, tag="sA" if i % 2 == 0 else "sB")
                    for kt in range(CT):
                        nc.tensor.matmul(vps, hts(kt, slice(i * P, (i + 1) * P)),
                                         wv[:, kt, :],
                                         start=(kt == 0), stop=(kt == CT - 1))
                    vv = V_g[:, i, :].rearrange("p (h d) -> p h d", h=4)
                    nc.vector.tensor_copy(
                        out=vv[:, :, 0:64],
                        in_=vps[:].rearrange("p (h d) -> p h d", h=4))
                    nc.vector.tensor_copy(out=vv[:, :, 64:65],
                                          in_=ones_col.to_broadcast((P, 4, 1)))
                for pr in range(2):
                    hp = 2 * pg + pr
                    # Q^T (own rows) / K^T (all rows) for this head pair
                    wq = stream.tile([P, CT, P], F32R, tag="w")
                    dma(out=wq, in_=w_qkv[:, hp * P:(hp + 1) * P]
                        .rearrange("(t p) j -> p t j", p=P))
                    qps = ps_b.tile([P, NO], F32, tag="sA")
                    for ch in range(2):
                        for kt in range(CT):
                            nc.tensor.matmul(qps[:, ch * 512:(ch + 1) * 512],
                                             wq[:, kt, :],
                                             hts(kt, slice(ch * 512, (ch + 1) * 512)),
                                             start=(kt == 0), stop=(kt == CT - 1))
                    QT = stream.tile([P, NO], F32R, tag="qt")
                    nc.vector.tensor_copy(QT, qps)
                    wk = stream.tile([P, CT, P], F32R, tag="w")
                    dma(out=wk, in_=w_qkv[:, C + hp * P:C + (hp + 1) * P]
                        .rearrange("(t p) j -> p t j", p=P))
                    KT = pkt.tile([P, N], F32R, tag="kt")
                    for half in range(2):
                        kps = ps_b.tile([P, NO], F32, tag="sA" if half == 0 else "sB")
                        for ch in range(2):
                            c0 = half * NO + ch * 512
                            for kt in range(CT):
                                nc.tensor.matmul(kps[:, ch * 512:(ch + 1) * 512],
                                                 wk[:, kt, :],
                                                 hts(kt, slice(c0, c0 + 512)),
                                                 start=(kt == 0), stop=(kt == CT - 1))
                        nc.vector.tensor_copy(KT[:, half * NO:(half + 1) * NO], kps)

                    yA = ps_y.tile([65, NO], F32, tag="yA")
                    yB = ps_y.tile([65, NO], F32, tag="yB")
                    for m in range(NT):
                        # separate per-head score tiles (separate PSUM banks ->
                        # the two row-group matmuls run concurrently, and exp of
                        # head A overlaps the QK matmuls of head B / tile m+1)
                        spsA = ps_b.tile([P, NO], F32, tag="sA")
                        spsB = ps_b.tile([P, NO], F32, tag="sB")
                        for ch in range(2):
                            nc.tensor.matmul(spsA[:, ch * 512:(ch + 1) * 512],
                                             KT[0:64, m * P:(m + 1) * P],
                                             QT[0:64, ch * 512:(ch + 1) * 512],
                                             start=True, stop=True,
                                             tile_position=(0, 0))
                        for ch in range(2):
                            nc.tensor.matmul(spsB[:, ch * 512:(ch + 1) * 512],
                                             KT[64:128, m * P:(m + 1) * P],
                                             QT[64:128, ch * 512:(ch + 1) * 512],
                                             start=True, stop=True,
                                             tile_position=(64, 0))
                        eA = stream.tile([P, NO], F32R, tag="e")
                        nc.scalar.activation(out=eA, in_=spsA[:], func=AF.Exp,
                                             scale=SCALE)
                        eB = stream.tile([P, NO], F32R, tag="e")
                        nc.scalar.activation(out=eB, in_=spsB[:],
                                             func=AF.Exp, scale=SCALE)
                        for ch in range(2):
                            nc.tensor.matmul(yA[:, ch * 512:(ch + 1) * 512],
                                             V_g[:, m, 65 * 2 * pr:65 * 2 * pr + 65],
                                             eA[:, ch * 512:(ch + 1) * 512],
                                             start=(m == 0), stop=(m == NT - 1))
                        for ch in range(2):
                            nc.tensor.matmul(yB[:, ch * 512:(ch + 1) * 512],
                                             V_g[:, m, 65 * (2 * pr + 1):65 * (2 * pr + 1) + 65],
                                             eB[:, ch * 512:(ch + 1) * 512],
                                             start=(m == 0), stop=(m == NT - 1))
                    # psum -> sbuf; odd head + denominators shift partitions by DMA
                    ytA = stream.tile([65, NO], F32, tag="e")
                    ytB = stream.tile([65, NO], F32, tag="e")
                    nc.vector.tensor_copy(out=YTraw[0:64, hp, :], in_=yA[0:64, :])
                    nc.vector.tensor_copy(out=ytA[64:65, :], in_=yA[64:65, :])
                    nc.vector.tensor_copy(ytB, yB)
                    dma(out=YTraw[64:128, hp, :], in_=ytB[0:64, :])
                    dma(out=den[2 * hp:2 * hp + 1, :], in_=ytA[64:65, :])
                    dma(out=den[2 * hp + 1:2 * hp + 2, :], in_=ytB[64:65, :])

        # ---------------- Phase C: normalize y by 1/den via diag-scaled transposes
        YTn = main.tile([P, CT, NO], F32R, tag="S4")
        rinv = psmall.tile([H, NO], F32R)
        with nc.allow_low_precision(reason="fp32r rounding of softmax denom"):
            nc.vector.reciprocal(out=rinv, in_=den)
        rT = psmall.tile([P, NOT_, H], F32)
        with tc.tile_pool(name="ps_n", bufs=3, space="PSUM") as ps_n:
            for i in range(NOT_):
                rtp = ps_n.tile([P, H], F32, tag="rT", bufs=2)
                nc.tensor.matmul(rtp, rinv[:, i * P:(i + 1) * P], ident_r[0:H, 0:H],
                                 start=True, stop=True)
                nc.vector.tensor_copy(out=rT[:, i, :], in_=rtp)
            for hp in range(CT):
                for i in range(NOT_):
                    # both heads of the pair -> y [token, dim] with tokens on
                    # partitions; 1/den rides the copy as a per-partition scalar
                    ysb2 = work.tile([P, P], F32, tag="ysb2")
                    for sub in range(2):
                        h, lo = 2 * hp + sub, 64 * sub
                        yps = ps_n.tile([P, 64], F32, tag="y_nt")
                        nc.tensor.transpose(yps,
                                            YTraw[lo:lo + 64, hp, i * P:(i + 1) * P],
                                            ident[lo:lo + 64, lo:lo + 64])
                        nc.vector.tensor_scalar(out=ysb2[:, lo:lo + 64], in0=yps,
                                                scalar1=rT[:, i, h:h + 1],
                                                scalar2=None, op0=A.mult)
                    ytp = ps_n.tile([P, P], F32, tag="yT_n")
                    nc.tensor.transpose(ytp, ysb2, ident)
                    nc.vector.tensor_copy(out=YTn[:, hp, i * P:(i + 1) * P],
                                          in_=ytp)

        # ---------------- Phase D: proj -> attnT (S5 slot)
        attnT = main.tile([P, CT, NO], F32, tag="S5")
        with tc.tile_pool(name="ps_p", bufs=4, space="PSUM") as ps_p:
            for cp in range(CT):
                wp = stream.tile([P, CT, P], F32R, tag="w")
                dma(out=wp, in_=w_proj[:, cp * P:(cp + 1) * P]
                    .rearrange("(t p) j -> p t j", p=P))
                pps = ps_p.tile([P, NO], F32, tag="p")
                for ch in range(2):
                    for kt in range(CT):
                        nc.tensor.matmul(pps[:, ch * 512:(ch + 1) * 512],
                                         wp[:, kt, :],
                                         YTn[:, kt, ch * 512:(ch + 1) * 512],
                                         start=(kt == 0), stop=(kt == CT - 1))
                nc.vector.tensor_scalar(out=attnT[:, cp, :], in0=pps,
                                        scalar1=bpT[:, cp:cp + 1], scalar2=None,
                                        op0=A.add)

        # ---------------- Phase E: attn + residual -> x2; LN2 -> x2lnT
        x2 = main.tile([P, NOT_, C], F32, tag="S4")
        x2lnT = main.tile([P, CT, NO], F32R, tag="S3")
        g2_bc = pbc.tile([P, C], F32, tag="g_bc")
        dma(out=g2_bc, in_=ln2_g[:].partition_broadcast(P))
        b2_bc = pbc.tile([P, C], F32, tag="b_bc")
        dma(out=b2_bc, in_=ln2_b[:].partition_broadcast(P))
        with tc.tile_pool(name="ps_trE", bufs=4, space="PSUM") as ps_trE:
            # sweep 1: attn^T -> attn, + residual -> x2 (PE + DVE pipeline)
            for i in range(NOT_):
                tp = ps_trE.tile([P, C], F32, tag="tr")
                for t in range(CT):
                    nc.tensor.transpose(tp[:, t * P:(t + 1) * P],
                                        attnT[:, t, i * P:(i + 1) * P], ident)
                xo = work.tile([P, C], F32, tag="io")
                dma(out=xo, in_=xb[i * P:(i + 1) * P, :])
                nc.vector.tensor_tensor(out=x2[:, i, :], in0=tp, in1=xo, op=A.add)
            # sweep 2: LN2 + transpose -> x2lnT
            for i in range(NOT_):
                hg2 = layernorm_tile(x2[:, i, :], g2_bc, b2_bc)
                transpose_768(hg2, x2lnT[:, :, i * P:(i + 1) * P], ps_trE)

        # ---------------- Phase F: MLP + residual + output, per 512-token half.
        # fc2 accumulates into six persistent PSUM banks as each gelu tile is
        # produced, so fc1/gelu/fc2 fully pipeline and no activation buffer is
        # needed in SBUF. w_fc2 row-slices load in natural [hid, c'] layout.
        for nh in range(2):
            sl = slice(nh * 512, (nh + 1) * 512)
            with tc.tile_pool(name="ps_mA%d" % nh, bufs=1, space="PSUM") as ps_mA:
                f2s = [ps_mA.tile([P, 512], F32, tag="f2c%d" % cp,
                                  name="f2acc%d_%d" % (nh, cp))
                       for cp in range(CT)]
                for ht in range(HT):
                    w1 = stream.tile([P, CT, P], F32R, tag="wf1", bufs=2)
                    dma(out=w1, in_=w_fc1[:, ht * P:(ht + 1) * P]
                        .rearrange("(t p) j -> p t j", p=P))
                    w2r = stream.tile([P, C], F32R, tag="wf2", bufs=2)
                    dma(out=w2r, in_=w_fc2[ht * P:(ht + 1) * P, :])
                    fps = ps_mA.tile([P, 512], F32,
                                     tag="f1a" if ht % 2 == 0 else "f1b")
                    for kt in range(CT):
                        nc.tensor.matmul(fps, w1[:, kt, :], x2lnT[:, kt, sl],
                                         start=(kt == 0), stop=(kt == CT - 1))
                    ga = work.tile([P, 512], F32R, tag="ga", bufs=3)
                    nc.scalar.activation(out=ga, in_=fps[:], func=AF.Gelu,
                                         bias=bf1T[:, ht:ht + 1])
                    for cp in range(CT):
                        nc.tensor.matmul(f2s[cp], w2r[:, cp * P:(cp + 1) * P], ga,
                                         start=(ht == 0), stop=(ht == HT - 1))
                mlpT = main.tile([P, CT, 512], F32, tag="S5")
                for cp in range(CT):
                    nc.vector.tensor_scalar(out=mlpT[:, cp, :], in0=f2s[cp],
                                            scalar1=bf2T[:, cp:cp + 1],
                                            scalar2=None, op0=A.add)
            with tc.tile_pool(name="ps_o%d" % nh, bufs=2, space="PSUM") as ps_o:
                for i in range(4):
                    it = nh * 4 + i
                    tp = ps_o.tile([P, C], F32, tag="tr")
                    for t in range(CT):
                        nc.tensor.transpose(tp[:, t * P:(t + 1) * P],
                                            mlpT[:, t, i * P:(i + 1) * P], ident)
                    o_sb = work.tile([P, C], F32, tag="io")
                    nc.vector.tensor_tensor(out=o_sb, in0=tp, in1=x2[:, it, :],
                                            op=A.add)
                    dma(out=out[it * P:(it + 1) * P, :], in_=o_sb)

        pkt.release()
        stream.release()
        main.release()
        work.release()
        psmall.release()
        pbc.release()
        consts.release()

    nc.compile()
    return nc


_NC_CACHE = None


def kernel(x, ln1_g, ln1_b, w_qkv, w_proj, b_proj, ln2_g, ln2_b,
           w_fc1, b_fc1, w_fc2, b_fc2):
    global _NC_CACHE
    from concourse.bass_utils import run_bass_kernel_spmd

    x = np.asarray(x, dtype=np.float32)
    shared = {
        "w_qkv": np.asarray(w_qkv, np.float32),
        "w_proj": np.asarray(w_proj, np.float32),
        "w_fc1": np.asarray(w_fc1, np.float32),
        "w_fc2": np.asarray(w_fc2, np.float32),
        "ln1_g": np.asarray(ln1_g, np.float32),
        "ln1_b": np.asarray(ln1_b, np.float32),
        "ln2_g": np.asarray(ln2_g, np.float32),
        "ln2_b": np.asarray(ln2_b, np.float32),
        "b_proj": np.asarray(b_proj, np.float32),
        "b_fc1": np.asarray(b_fc1, np.float32),
        "b_fc2": np.asarray(b_fc2, np.float32),
    }
    in_maps = []
    for c in range(8):
        b, h = c // 2, c % 2
        xbv = np.ascontiguousarray(np.roll(x[b], -h * NO, axis=0))
        in_maps.append({"xb": xbv, **shared})

    if _NC_CACHE is None:
        _NC_CACHE = _build_bass()
    res = run_bass_kernel_spmd(_NC_CACHE, in_maps, core_ids=list(range(8)))

    outp = np.empty((B, N, C), np.float32)
    for c in range(8):
        b, h = c // 2, c % 2
        outp[b, h * NO:(h + 1) * NO, :] = res.results[c]["out"]
    return outp
